# revision 4
# baseline (speedup 1.0000x reference)
"""Bass/Trainium2 kernel v3 for nn_LogReg_8151847928094.

out[b] = sum_s w[text[s, b]] + bias   (bag-of-words logistic regression)

Gather-as-local_scatter design. gpsimd local_scatter streams (data, idx)
pairs from SBUF and uses the Q7 hardware scatter into local RAM
(~3.7 ns/entry) -- ~8x cheaper per element than ap_gather's
read-command-per-4-indices structure.

  - Token v = hi*128 + lo. The bf16 table has K=8 rotated regions:
    T[p, k*HI + h] = w[h*128 + (p + 17k) % 128]. Token (hi, lo) appears
    in region k on partition p_k = (lo - 17k) % 128 at entry k*HI + hi.
    (Stride 17, not 16: the K shifts then span different mod-16 classes,
    so a column's tokens spread over 8*K candidate bins, not 8.)
  - Host picks a rotation k per token: same-value tokens in a core take
    distinct k (a table entry may feed only ONE output slot), and a
    vectorized repair pass keeps every (partition, column) bin <= CAP.
  - idxs[p, e] = destination slot (col*CAP + j) for the token served by
    table entry e, or -1 for the ~94% of entries that serve no token.
  - Device: local_scatter -> dst [128, 256*CAP] bf16 (pad slots auto-
    zeroed), DVE segmented reduce -> [128, 256] f32, PE ones-matmul
    partition-sum -> [1, 256], add bias, DMA out.
"""

import sys

sys.path.insert(0, "/opt/trn_rl_repo")

import numpy as np
import ml_dtypes

import concourse.bass as bass
import concourse.bacc as bacc
import concourse.mybir as mybir
import concourse.tile as tile
from concourse.bass_utils import run_bass_kernel_spmd

S = 200
B = 2048
V = 100000
NCORES = 8
BS = B // NCORES  # 256 batch columns per core
P = 128
HI = (V + P - 1) // P  # 782
SHIFT = 17  # rotation stride; odd so the K shifts cover all mod-16 classes

_prog_cache = {}


def _build_program(K, CAP, loop_T=None):
    NIDXS = K * HI
    NELEM = BS * CAP
    assert NELEM % 2 == 0 and NELEM * 32 < 2**16
    nc = bacc.Bacc("TRN2", target_bir_lowering=False, debug=False)
    data_d = nc.declare_dram_parameter("data", [P, NIDXS], mybir.dt.bfloat16, isOutput=False)
    idxs_d = nc.declare_dram_parameter("idxs", [P, NIDXS], mybir.dt.int16, isOutput=False)
    bias_d = nc.declare_dram_parameter("bias", [1, BS], mybir.dt.float32, isOutput=False)
    out_d = nc.declare_dram_parameter("out", [1, BS], mybir.dt.float32, isOutput=True)

    with tile.TileContext(nc) as tc:
        with (
            tc.tile_pool(name="sbuf", bufs=2) as pool,
            tc.tile_pool(name="psum", bufs=2, space="PSUM") as psum_pool,
        ):

            def body():
                # allocated per loop iteration so the pool double-buffers:
                # iteration i+1's DMAs overlap iteration i's scatter/reduce
                data_t = pool.tile([P, NIDXS], mybir.dt.bfloat16)
                idxs_t = pool.tile([P, NIDXS], mybir.dt.int16)
                dst_t = pool.tile([P, NELEM], mybir.dt.bfloat16)
                red_t = pool.tile([P, BS], mybir.dt.float32)
                ones_t = pool.tile([P, 1], mybir.dt.float32)
                bias_t = pool.tile([1, BS], mybir.dt.float32)
                res_t = pool.tile([1, BS], mybir.dt.float32)
                psum_t = psum_pool.tile([1, BS], mybir.dt.float32)
                nc.gpsimd.memset(ones_t[:], 1.0)
                nc.sync.dma_start(out=data_t[:], in_=data_d[:])
                nc.sync.dma_start(out=idxs_t[:], in_=idxs_d[:])
                nc.sync.dma_start(out=bias_t[:], in_=bias_d[:])
                nc.gpsimd.local_scatter(
                    dst_t[:],
                    data_t[:],
                    idxs_t[:],
                    channels=P,
                    num_elems=NELEM,
                    num_idxs=NIDXS,
                )
                nc.vector.tensor_reduce(
                    out=red_t[:],
                    in_=dst_t[:].rearrange("p (b j) -> p b j", j=CAP),
                    axis=mybir.AxisListType.X,
                    op=mybir.AluOpType.add,
                )
                nc.tensor.matmul(
                    psum_t[:], lhsT=ones_t[:], rhs=red_t[:], start=True, stop=True
                )
                nc.vector.tensor_tensor(
                    out=res_t[:], in0=psum_t[:], in1=bias_t[:], op=mybir.AluOpType.add
                )
                nc.sync.dma_start(out=out_d[:], in_=res_t[:])

            if loop_T is None:
                body()
            else:
                with tc.For_i(0, loop_T, 1) as _i:
                    body()
    nc.compile()
    return nc


def _build_table(w, K):
    """[P, K*HI] bf16: region k col h partition p = w[h*128 + (p+16k)%128]."""
    w_pad = np.zeros(HI * P, np.float32)
    w_pad[:V] = w
    wr = w_pad.reshape(HI, P)  # wr[h, lo]
    tbl = np.empty((P, K * HI), np.float32)
    p = np.arange(P)
    for k in range(K):
        tbl[:, k * HI : (k + 1) * HI] = wr[:, (p + SHIFT * k) % P].T
    return tbl.astype(ml_dtypes.bfloat16)


def _assign_core(tokens, K, CAP):
    """tokens [S, BS] -> idxs [P, K*HI] int16 scatter map, or None if the
    (K, CAP) configuration cannot host this input."""
    t = tokens.astype(np.int64)
    v = t.ravel()  # token values, s-major
    c = np.broadcast_to(np.arange(BS)[None, :], (S, BS)).ravel().astype(np.int64)
    lo = v % P
    hi = v // P

    # value-group ranks (same-value tokens must take distinct rotations)
    order = np.argsort(v, kind="stable")
    vs = v[order]
    newgrp = np.r_[True, np.diff(vs) != 0]
    gid_sorted = np.cumsum(newgrp) - 1
    starts = np.flatnonzero(newgrp)
    rank_sorted = np.arange(v.size) - starts[gid_sorted]
    rank = np.empty(v.size, np.int64)
    gid = np.empty(v.size, np.int64)
    rank[order] = rank_sorted
    gid[order] = gid_sorted
    ngroups = starts.size
    if rank.max() >= K:
        return None

    # initial rotation: distinct within group by construction
    k = (rank + v % K) % K
    p = (lo - SHIFT * k) % P
    bins = p * BS + c
    loads = np.bincount(bins, minlength=P * BS)

    used = np.zeros(ngroups, np.uint16)
    np.bitwise_or.at(used, gid, (1 << k).astype(np.uint16))

    # repair over-cap (partition, column) bins
    for _ in range(20):
        movers = np.flatnonzero(loads[bins] > CAP)
        if movers.size == 0:
            break
        for tk in movers:
            if loads[bins[tk]] <= CAP:
                continue
            gi = gid[tk]
            best_k, best_load = -1, None
            for kk in range(K):
                if used[gi] & (1 << kk):
                    continue
                pb = int((lo[tk] - SHIFT * kk) % P) * BS + int(c[tk])
                if loads[pb] < CAP and (best_load is None or loads[pb] < best_load):
                    best_k, best_load = kk, loads[pb]
            if best_k < 0:
                continue
            used[gi] &= np.uint16(~np.uint16(1 << k[tk]) & 0xFFFF)
            used[gi] |= np.uint16(1 << best_k)
            loads[bins[tk]] -= 1
            k[tk] = best_k
            p[tk] = (lo[tk] - SHIFT * best_k) % P
            bins[tk] = p[tk] * BS + c[tk]
            loads[bins[tk]] += 1
    else:
        pass
    if (np.bincount(bins, minlength=P * BS) > CAP).any():
        return None

    # j-slot within each (p, c) bin
    order2 = np.argsort(bins, kind="stable")
    bs2 = bins[order2]
    newbin = np.r_[True, np.diff(bs2) != 0]
    bstarts = np.flatnonzero(newbin)
    bid = np.cumsum(newbin) - 1
    j_sorted = np.arange(v.size) - bstarts[bid]
    j = np.empty(v.size, np.int64)
    j[order2] = j_sorted
    assert j.max() < CAP

    idxs = np.full((P, K * HI), -1, np.int16)
    entry = k * HI + hi
    if np.unique(p * (K * HI) + entry).size != v.size:
        return None  # two tokens claimed one table entry; try a bigger config
    slot = c * CAP + j
    idxs[p, entry] = slot.astype(np.int16)
    return idxs


def kernel(text, w, b):
    text = np.asarray(text)
    w = np.asarray(w, dtype=np.float32).reshape(-1)
    b = np.asarray(b, dtype=np.float32).reshape(-1)

    for K, CAP in ((6, 4), (6, 5), (6, 6), (7, 5), (8, 4), (8, 5), (10, 5), (12, 7), (16, 7)):
        idx_maps = []
        for c in range(NCORES):
            m = _assign_core(text[:, c * BS : (c + 1) * BS], K, CAP)
            if m is None:
                break
            idx_maps.append(m)
        if len(idx_maps) == NCORES:
            break
    else:
        raise RuntimeError("no feasible (K, CAP) config")

    nc = _prog_cache.get((K, CAP))
    if nc is None:
        nc = _build_program(K, CAP)
        _prog_cache[(K, CAP)] = nc

    tbl = _build_table(w, K)
    bias_row = np.full((1, BS), b[0], np.float32)
    in_maps = [
        {"data": tbl, "idxs": idx_maps[c], "bias": bias_row} for c in range(NCORES)
    ]

    res = run_bass_kernel_spmd(nc, in_maps, list(range(NCORES))).results
    out = np.concatenate([res[c]["out"][0] for c in range(NCORES)])
    return out.astype(np.float32)


def _sim_core(tokens, w, bval, K=6, CAP=6):
    """Numpy simulation of the device program for one core."""
    idxs = _assign_core(tokens, K, CAP)
    assert idxs is not None
    tbl = _build_table(w, K).astype(np.float32)
    dst = np.zeros((P, BS * CAP), np.float32)
    for p in range(P):
        m = idxs[p] >= 0
        dst[p, idxs[p, m]] = tbl[p, m]
    red = dst.reshape(P, BS, CAP).sum(axis=2)
    return (red.sum(axis=0) + bval).astype(np.float32)


if __name__ == "__main__":
    rng = np.random.default_rng(0)
    text = rng.integers(0, V, (S, B)).astype(np.int64)
    w = rng.standard_normal(V).astype(np.float32) * 0.01
    bval = 0.125
    exp = w[text].sum(axis=0) + bval
    sim = np.concatenate(
        [_sim_core(text[:, c * BS : (c + 1) * BS], w, bval) for c in range(NCORES)]
    )
    err = np.abs(sim - exp).max() / (np.abs(exp).max() + 1e-9)
    print("sim rel err:", err)
    assert err < 2e-2, "sim failed"
    out = kernel(text, w.reshape(1, -1), np.array([bval], np.float32))
    err = np.abs(out - exp).max() / (np.abs(exp).max() + 1e-9)
    print("hw rel err:", err)


# revision 5
# speedup vs baseline: 1.6092x; 1.6092x over previous
"""Bass/Trainium2 kernel v3 for nn_LogReg_8151847928094.

out[b] = sum_s w[text[s, b]] + bias   (bag-of-words logistic regression)

Gather-as-local_scatter design. gpsimd local_scatter streams (data, idx)
pairs from SBUF and uses the Q7 hardware scatter into local RAM
(~3.7 ns/entry) -- ~8x cheaper per element than ap_gather's
read-command-per-4-indices structure.

  - Token v = hi*128 + lo. The bf16 table has K=8 rotated regions:
    T[p, k*HI + h] = w[h*128 + (p + 17k) % 128]. Token (hi, lo) appears
    in region k on partition p_k = (lo - 17k) % 128 at entry k*HI + hi.
    (Stride 17, not 16: the K shifts then span different mod-16 classes,
    so a column's tokens spread over 8*K candidate bins, not 8.)
  - Host picks a rotation k per token: same-value tokens in a core take
    distinct k (a table entry may feed only ONE output slot), and a
    vectorized repair pass keeps every (partition, column) bin <= CAP.
  - idxs[p, e] = destination slot (col*CAP + j) for the token served by
    table entry e, or -1 for the ~94% of entries that serve no token.
  - Device: local_scatter -> dst [128, 256*CAP] bf16 (pad slots auto-
    zeroed), DVE segmented reduce -> [128, 256] f32, PE ones-matmul
    partition-sum -> [1, 256], add bias, DMA out.
"""

import sys

sys.path.insert(0, "/opt/trn_rl_repo")

import numpy as np
import ml_dtypes

import concourse.bass as bass
import concourse.bacc as bacc
import concourse.mybir as mybir
import concourse.tile as tile
from concourse.bass_utils import run_bass_kernel_spmd

S = 200
B = 2048
V = 100000
NCORES = 8
BS = B // NCORES  # 256 batch columns per core
P = 128
HI = (V + P - 1) // P  # 782
SHIFT = 17  # rotation stride; odd so the K shifts cover all mod-16 classes

_prog_cache = {}


def _build_program(K, CAP, loop_T=None):
    NIDXS = K * HI
    NELEM = BS * CAP
    assert NELEM % 2 == 0 and NELEM * 32 < 2**16
    nc = bacc.Bacc("TRN2", target_bir_lowering=False, debug=False)
    data_d = nc.declare_dram_parameter("data", [P, NIDXS], mybir.dt.bfloat16, isOutput=False)
    idxs_d = nc.declare_dram_parameter("idxs", [P, NIDXS], mybir.dt.int16, isOutput=False)
    bias_d = nc.declare_dram_parameter("bias", [1, BS], mybir.dt.float32, isOutput=False)
    out_d = nc.declare_dram_parameter("out", [1, BS], mybir.dt.float32, isOutput=True)

    with tile.TileContext(nc) as tc:
        with (
            tc.tile_pool(name="sbuf", bufs=3) as pool,
            tc.tile_pool(name="fixed", bufs=1) as fpool,
            tc.tile_pool(name="psum", bufs=2, space="PSUM") as psum_pool,
        ):
            ones_t = fpool.tile([P, 1], mybir.dt.float32)
            nc.gpsimd.memset(ones_t[:], 1.0)

            def body():
                # allocated per loop iteration so the pool multi-buffers:
                # later iterations' DMAs overlap this iteration's scatter.
                # The two 1.2MB loads go on different HWDGE rings (SP for
                # idxs, Activation for data) so they transfer in parallel.
                data_t = pool.tile([P, NIDXS], mybir.dt.bfloat16)
                idxs_t = pool.tile([P, NIDXS], mybir.dt.int16)
                dst_t = pool.tile([P, NELEM], mybir.dt.bfloat16)
                red_t = pool.tile([P, BS], mybir.dt.float32)
                bias_t = pool.tile([1, BS], mybir.dt.float32)
                res_t = pool.tile([1, BS], mybir.dt.float32)
                psum_t = psum_pool.tile([1, BS], mybir.dt.float32)
                nc.scalar.dma_start(out=data_t[:], in_=data_d[:])
                nc.sync.dma_start(out=idxs_t[:], in_=idxs_d[:])
                nc.scalar.dma_start(out=bias_t[:], in_=bias_d[:])
                nc.gpsimd.local_scatter(
                    dst_t[:],
                    data_t[:],
                    idxs_t[:],
                    channels=P,
                    num_elems=NELEM,
                    num_idxs=NIDXS,
                )
                nc.vector.tensor_reduce(
                    out=red_t[:],
                    in_=dst_t[:].rearrange("p (b j) -> p b j", j=CAP),
                    axis=mybir.AxisListType.X,
                    op=mybir.AluOpType.add,
                )
                nc.tensor.matmul(
                    psum_t[:], lhsT=ones_t[:], rhs=red_t[:], start=True, stop=True
                )
                nc.vector.tensor_tensor(
                    out=res_t[:], in0=psum_t[:], in1=bias_t[:], op=mybir.AluOpType.add
                )
                nc.sync.dma_start(out=out_d[:], in_=res_t[:])

            if loop_T is None:
                body()
            else:
                with tc.For_i(0, loop_T, 1) as _i:
                    body()
    nc.compile()
    return nc


def _build_table(w, K):
    """[P, K*HI] bf16: region k col h partition p = w[h*128 + (p+16k)%128]."""
    w_pad = np.zeros(HI * P, np.float32)
    w_pad[:V] = w
    wr = w_pad.reshape(HI, P)  # wr[h, lo]
    tbl = np.empty((P, K * HI), np.float32)
    p = np.arange(P)
    for k in range(K):
        tbl[:, k * HI : (k + 1) * HI] = wr[:, (p + SHIFT * k) % P].T
    return tbl.astype(ml_dtypes.bfloat16)


def _assign_core(tokens, K, CAP):
    """tokens [S, BS] -> idxs [P, K*HI] int16 scatter map, or None if the
    (K, CAP) configuration cannot host this input."""
    t = tokens.astype(np.int64)
    v = t.ravel()  # token values, s-major
    c = np.broadcast_to(np.arange(BS)[None, :], (S, BS)).ravel().astype(np.int64)
    lo = v % P
    hi = v // P

    # value-group ranks (same-value tokens must take distinct rotations)
    order = np.argsort(v, kind="stable")
    vs = v[order]
    newgrp = np.r_[True, np.diff(vs) != 0]
    gid_sorted = np.cumsum(newgrp) - 1
    starts = np.flatnonzero(newgrp)
    rank_sorted = np.arange(v.size) - starts[gid_sorted]
    rank = np.empty(v.size, np.int64)
    gid = np.empty(v.size, np.int64)
    rank[order] = rank_sorted
    gid[order] = gid_sorted
    ngroups = starts.size
    if rank.max() >= K:
        return None

    # initial rotation: distinct within group by construction
    k = (rank + v % K) % K
    p = (lo - SHIFT * k) % P
    bins = p * BS + c
    loads = np.bincount(bins, minlength=P * BS)

    used = np.zeros(ngroups, np.uint16)
    np.bitwise_or.at(used, gid, (1 << k).astype(np.uint16))

    # repair over-cap (partition, column) bins
    for _ in range(20):
        movers = np.flatnonzero(loads[bins] > CAP)
        if movers.size == 0:
            break
        for tk in movers:
            if loads[bins[tk]] <= CAP:
                continue
            gi = gid[tk]
            best_k, best_load = -1, None
            for kk in range(K):
                if used[gi] & (1 << kk):
                    continue
                pb = int((lo[tk] - SHIFT * kk) % P) * BS + int(c[tk])
                if loads[pb] < CAP and (best_load is None or loads[pb] < best_load):
                    best_k, best_load = kk, loads[pb]
            if best_k < 0:
                continue
            used[gi] &= np.uint16(~np.uint16(1 << k[tk]) & 0xFFFF)
            used[gi] |= np.uint16(1 << best_k)
            loads[bins[tk]] -= 1
            k[tk] = best_k
            p[tk] = (lo[tk] - SHIFT * best_k) % P
            bins[tk] = p[tk] * BS + c[tk]
            loads[bins[tk]] += 1
    else:
        pass
    if (np.bincount(bins, minlength=P * BS) > CAP).any():
        return None

    # j-slot within each (p, c) bin
    order2 = np.argsort(bins, kind="stable")
    bs2 = bins[order2]
    newbin = np.r_[True, np.diff(bs2) != 0]
    bstarts = np.flatnonzero(newbin)
    bid = np.cumsum(newbin) - 1
    j_sorted = np.arange(v.size) - bstarts[bid]
    j = np.empty(v.size, np.int64)
    j[order2] = j_sorted
    assert j.max() < CAP

    idxs = np.full((P, K * HI), -1, np.int16)
    entry = k * HI + hi
    if np.unique(p * (K * HI) + entry).size != v.size:
        return None  # two tokens claimed one table entry; try a bigger config
    slot = c * CAP + j
    idxs[p, entry] = slot.astype(np.int16)
    return idxs


def kernel(text, w, b):
    text = np.asarray(text)
    w = np.asarray(w, dtype=np.float32).reshape(-1)
    b = np.asarray(b, dtype=np.float32).reshape(-1)

    for K, CAP in ((6, 4), (6, 5), (6, 6), (7, 5), (8, 4), (8, 5), (10, 5), (12, 7), (16, 7)):
        idx_maps = []
        for c in range(NCORES):
            m = _assign_core(text[:, c * BS : (c + 1) * BS], K, CAP)
            if m is None:
                break
            idx_maps.append(m)
        if len(idx_maps) == NCORES:
            break
    else:
        raise RuntimeError("no feasible (K, CAP) config")

    nc = _prog_cache.get((K, CAP))
    if nc is None:
        nc = _build_program(K, CAP)
        _prog_cache[(K, CAP)] = nc

    tbl = _build_table(w, K)
    bias_row = np.full((1, BS), b[0], np.float32)
    in_maps = [
        {"data": tbl, "idxs": idx_maps[c], "bias": bias_row} for c in range(NCORES)
    ]

    res = run_bass_kernel_spmd(nc, in_maps, list(range(NCORES))).results
    out = np.concatenate([res[c]["out"][0] for c in range(NCORES)])
    return out.astype(np.float32)


def _sim_core(tokens, w, bval, K=6, CAP=6):
    """Numpy simulation of the device program for one core."""
    idxs = _assign_core(tokens, K, CAP)
    assert idxs is not None
    tbl = _build_table(w, K).astype(np.float32)
    dst = np.zeros((P, BS * CAP), np.float32)
    for p in range(P):
        m = idxs[p] >= 0
        dst[p, idxs[p, m]] = tbl[p, m]
    red = dst.reshape(P, BS, CAP).sum(axis=2)
    return (red.sum(axis=0) + bval).astype(np.float32)


if __name__ == "__main__":
    rng = np.random.default_rng(0)
    text = rng.integers(0, V, (S, B)).astype(np.int64)
    w = rng.standard_normal(V).astype(np.float32) * 0.01
    bval = 0.125
    exp = w[text].sum(axis=0) + bval
    sim = np.concatenate(
        [_sim_core(text[:, c * BS : (c + 1) * BS], w, bval) for c in range(NCORES)]
    )
    err = np.abs(sim - exp).max() / (np.abs(exp).max() + 1e-9)
    print("sim rel err:", err)
    assert err < 2e-2, "sim failed"
    out = kernel(text, w.reshape(1, -1), np.array([bval], np.float32))
    err = np.abs(out - exp).max() / (np.abs(exp).max() + 1e-9)
    print("hw rel err:", err)


# revision 6
# speedup vs baseline: 2.7137x; 1.6864x over previous
"""Bass/Trainium2 kernel v3 for nn_LogReg_8151847928094.

out[b] = sum_s w[text[s, b]] + bias   (bag-of-words logistic regression)

Gather-as-local_scatter design. gpsimd local_scatter streams (data, idx)
pairs from SBUF and uses the Q7 hardware scatter into local RAM
(~3.7 ns/entry) -- ~8x cheaper per element than ap_gather's
read-command-per-4-indices structure.

  - Token v = hi*128 + lo. The bf16 table has K=8 rotated regions:
    T[p, k*HI + h] = w[h*128 + (p + 17k) % 128]. Token (hi, lo) appears
    in region k on partition p_k = (lo - 17k) % 128 at entry k*HI + hi.
    (Stride 17, not 16: the K shifts then span different mod-16 classes,
    so a column's tokens spread over 8*K candidate bins, not 8.)
  - Host picks a rotation k per token: same-value tokens in a core take
    distinct k (a table entry may feed only ONE output slot), and a
    vectorized repair pass keeps every (partition, column) bin <= CAP.
  - idxs[p, e] = destination slot (col*CAP + j) for the token served by
    table entry e, or -1 for the ~94% of entries that serve no token.
  - Device: local_scatter -> dst [128, 256*CAP] bf16 (pad slots auto-
    zeroed), DVE segmented reduce -> [128, 256] f32, PE ones-matmul
    partition-sum -> [1, 256], add bias, DMA out.
"""

import sys

sys.path.insert(0, "/opt/trn_rl_repo")

import numpy as np
import ml_dtypes

import concourse.bass as bass
import concourse.bacc as bacc
import concourse.mybir as mybir
import concourse.tile as tile
from concourse.bass_utils import run_bass_kernel_spmd

S = 200
B = 2048
V = 100000
NCORES = 8
BS = B // NCORES  # 256 batch columns per core
P = 128
HI = (V + P - 1) // P  # 782
SHIFT = 17  # rotation stride; odd so the K shifts cover all mod-16 classes

_prog_cache = {}


def _build_program(K, CAP, loop_T=None):
    NIDXS = K * HI
    NELEM = BS * CAP
    assert NELEM % 2 == 0 and NELEM * 32 < 2**16
    nc = bacc.Bacc("TRN2", target_bir_lowering=False, debug=False)
    data_d = nc.declare_dram_parameter("data", [P, NIDXS], mybir.dt.bfloat16, isOutput=False)
    idxs_d = nc.declare_dram_parameter("idxs", [P, NIDXS], mybir.dt.int16, isOutput=False)
    bias_d = nc.declare_dram_parameter("bias", [1, BS], mybir.dt.float32, isOutput=False)
    out_d = nc.declare_dram_parameter("out", [1, BS], mybir.dt.float32, isOutput=True)

    with tile.TileContext(nc) as tc:
        with (
            tc.tile_pool(name="sbuf", bufs=3) as pool,
            tc.tile_pool(name="fixed", bufs=1) as fpool,
            tc.tile_pool(name="psum", bufs=2, space="PSUM") as psum_pool,
        ):
            ones_t = fpool.tile([P, 1], mybir.dt.float32)
            nc.gpsimd.memset(ones_t[:], 1.0)

            def body():
                # allocated per loop iteration so the pool multi-buffers:
                # later iterations' DMAs overlap this iteration's scatter.
                # The two 1.2MB loads go on different HWDGE rings (SP for
                # idxs, Activation for data) so they transfer in parallel.
                data_t = pool.tile([P, NIDXS], mybir.dt.bfloat16)
                idxs_t = pool.tile([P, NIDXS], mybir.dt.int16)
                dst_t = pool.tile([P, NELEM], mybir.dt.bfloat16)
                red_t = pool.tile([P, BS], mybir.dt.float32)
                bias_t = pool.tile([1, BS], mybir.dt.float32)
                res_t = pool.tile([1, BS], mybir.dt.float32)
                psum_t = psum_pool.tile([1, BS], mybir.dt.float32)
                nc.scalar.dma_start(out=data_t[:], in_=data_d[:])
                nc.sync.dma_start(out=idxs_t[:], in_=idxs_d[:])
                nc.scalar.dma_start(out=bias_t[:], in_=bias_d[:])
                nc.gpsimd.local_scatter(
                    dst_t[:],
                    data_t[:],
                    idxs_t[:],
                    channels=P,
                    num_elems=NELEM,
                    num_idxs=NIDXS,
                )
                nc.vector.tensor_reduce(
                    out=red_t[:],
                    in_=dst_t[:].rearrange("p (b j) -> p b j", j=CAP),
                    axis=mybir.AxisListType.X,
                    op=mybir.AluOpType.add,
                )
                nc.tensor.matmul(
                    psum_t[:], lhsT=ones_t[:], rhs=red_t[:], start=True, stop=True
                )
                nc.vector.tensor_tensor(
                    out=res_t[:], in0=psum_t[:], in1=bias_t[:], op=mybir.AluOpType.add
                )
                nc.sync.dma_start(out=out_d[:], in_=res_t[:])

            if loop_T is None:
                body()
            else:
                # unroll to amortize per-iteration loop/sync overhead
                unroll = 4 if loop_T % 4 == 0 else 1
                with tc.For_i(0, loop_T // unroll, 1) as _i:
                    for _ in range(unroll):
                        body()
    nc.compile()
    return nc


def _build_table(w, K):
    """[P, K*HI] bf16: region k col h partition p = w[h*128 + (p+16k)%128]."""
    w_pad = np.zeros(HI * P, np.float32)
    w_pad[:V] = w
    wr = w_pad.reshape(HI, P)  # wr[h, lo]
    tbl = np.empty((P, K * HI), np.float32)
    p = np.arange(P)
    for k in range(K):
        tbl[:, k * HI : (k + 1) * HI] = wr[:, (p + SHIFT * k) % P].T
    return tbl.astype(ml_dtypes.bfloat16)


def _assign_core(tokens, K, CAP):
    """tokens [S, BS] -> idxs [P, K*HI] int16 scatter map, or None if the
    (K, CAP) configuration cannot host this input."""
    t = tokens.astype(np.int64)
    v = t.ravel()  # token values, s-major
    c = np.broadcast_to(np.arange(BS)[None, :], (S, BS)).ravel().astype(np.int64)
    lo = v % P
    hi = v // P

    # value-group ranks (same-value tokens must take distinct rotations)
    order = np.argsort(v, kind="stable")
    vs = v[order]
    newgrp = np.r_[True, np.diff(vs) != 0]
    gid_sorted = np.cumsum(newgrp) - 1
    starts = np.flatnonzero(newgrp)
    rank_sorted = np.arange(v.size) - starts[gid_sorted]
    rank = np.empty(v.size, np.int64)
    gid = np.empty(v.size, np.int64)
    rank[order] = rank_sorted
    gid[order] = gid_sorted
    ngroups = starts.size
    if rank.max() >= K:
        return None

    # initial rotation: distinct within group by construction
    k = (rank + v % K) % K
    p = (lo - SHIFT * k) % P
    bins = p * BS + c
    loads = np.bincount(bins, minlength=P * BS)

    used = np.zeros(ngroups, np.uint16)
    np.bitwise_or.at(used, gid, (1 << k).astype(np.uint16))

    # repair over-cap (partition, column) bins
    for _ in range(20):
        movers = np.flatnonzero(loads[bins] > CAP)
        if movers.size == 0:
            break
        for tk in movers:
            if loads[bins[tk]] <= CAP:
                continue
            gi = gid[tk]
            best_k, best_load = -1, None
            for kk in range(K):
                if used[gi] & (1 << kk):
                    continue
                pb = int((lo[tk] - SHIFT * kk) % P) * BS + int(c[tk])
                if loads[pb] < CAP and (best_load is None or loads[pb] < best_load):
                    best_k, best_load = kk, loads[pb]
            if best_k < 0:
                continue
            used[gi] &= np.uint16(~np.uint16(1 << k[tk]) & 0xFFFF)
            used[gi] |= np.uint16(1 << best_k)
            loads[bins[tk]] -= 1
            k[tk] = best_k
            p[tk] = (lo[tk] - SHIFT * best_k) % P
            bins[tk] = p[tk] * BS + c[tk]
            loads[bins[tk]] += 1
    else:
        pass
    if (np.bincount(bins, minlength=P * BS) > CAP).any():
        return None

    # j-slot within each (p, c) bin
    order2 = np.argsort(bins, kind="stable")
    bs2 = bins[order2]
    newbin = np.r_[True, np.diff(bs2) != 0]
    bstarts = np.flatnonzero(newbin)
    bid = np.cumsum(newbin) - 1
    j_sorted = np.arange(v.size) - bstarts[bid]
    j = np.empty(v.size, np.int64)
    j[order2] = j_sorted
    assert j.max() < CAP

    idxs = np.full((P, K * HI), -1, np.int16)
    entry = k * HI + hi
    if np.unique(p * (K * HI) + entry).size != v.size:
        return None  # two tokens claimed one table entry; try a bigger config
    slot = c * CAP + j
    idxs[p, entry] = slot.astype(np.int16)
    return idxs


def kernel(text, w, b):
    text = np.asarray(text)
    w = np.asarray(w, dtype=np.float32).reshape(-1)
    b = np.asarray(b, dtype=np.float32).reshape(-1)

    for K, CAP in ((6, 4), (6, 5), (6, 6), (7, 5), (8, 4), (8, 5), (10, 5), (12, 7), (16, 7)):
        idx_maps = []
        for c in range(NCORES):
            m = _assign_core(text[:, c * BS : (c + 1) * BS], K, CAP)
            if m is None:
                break
            idx_maps.append(m)
        if len(idx_maps) == NCORES:
            break
    else:
        raise RuntimeError("no feasible (K, CAP) config")

    nc = _prog_cache.get((K, CAP))
    if nc is None:
        nc = _build_program(K, CAP)
        _prog_cache[(K, CAP)] = nc

    tbl = _build_table(w, K)
    bias_row = np.full((1, BS), b[0], np.float32)
    in_maps = [
        {"data": tbl, "idxs": idx_maps[c], "bias": bias_row} for c in range(NCORES)
    ]

    res = run_bass_kernel_spmd(nc, in_maps, list(range(NCORES))).results
    out = np.concatenate([res[c]["out"][0] for c in range(NCORES)])
    return out.astype(np.float32)


def _sim_core(tokens, w, bval, K=6, CAP=6):
    """Numpy simulation of the device program for one core."""
    idxs = _assign_core(tokens, K, CAP)
    assert idxs is not None
    tbl = _build_table(w, K).astype(np.float32)
    dst = np.zeros((P, BS * CAP), np.float32)
    for p in range(P):
        m = idxs[p] >= 0
        dst[p, idxs[p, m]] = tbl[p, m]
    red = dst.reshape(P, BS, CAP).sum(axis=2)
    return (red.sum(axis=0) + bval).astype(np.float32)


if __name__ == "__main__":
    rng = np.random.default_rng(0)
    text = rng.integers(0, V, (S, B)).astype(np.int64)
    w = rng.standard_normal(V).astype(np.float32) * 0.01
    bval = 0.125
    exp = w[text].sum(axis=0) + bval
    sim = np.concatenate(
        [_sim_core(text[:, c * BS : (c + 1) * BS], w, bval) for c in range(NCORES)]
    )
    err = np.abs(sim - exp).max() / (np.abs(exp).max() + 1e-9)
    print("sim rel err:", err)
    assert err < 2e-2, "sim failed"
    out = kernel(text, w.reshape(1, -1), np.array([bval], np.float32))
    err = np.abs(out - exp).max() / (np.abs(exp).max() + 1e-9)
    print("hw rel err:", err)
